# revision 54
# baseline (speedup 1.0000x reference)
"""BoundaryLoss (EDT-weighted BCE) on 8 Trainium2 NeuronCores.

Math: reference computes dist_pos = EDT(target), dist_neg = EDT(1-target),
a = |dist_neg - dist_pos| (= dist_pos + dist_neg since one of them is 0 at
every voxel of a binary mask), w = clamp(1 - (a-3)/2, 0, 1), and the
w-weighted per-sample-normalized BCE mean.

w only depends on min(a, 5): any voxel farther than 5 from the opposite
class has w = 0. Squared grid distances are integers, so the relevant
squared distances are <= 24, and every separable squared-distance-transform
pass only needs a +-4 window:
    g[i] = min_{|d|<=4} (d^2 + f[i+d])
Values that are truly >= 25 stay >= 25 under the windowed min (g >= g_true)
and values < 25 are computed exactly, so sqrt + the w ramp are exact. The
passes commute, so they run W (trims to the 24 interior columns and absorbs
the f-init), then H (DMA-shifted copies), then D, whose last step is split
into D-halves so the finalization pipelines with the end of the pass.

Sharding: 8 slabs = 2 batches x 4 W-quarters (24 cols each + 4-col halo,
edge-replicated at the volume boundary -- replicated-column contributions
are always dominated by the real column's, so the result is exact). The D
and H passes never mix W columns, so halo columns are exact after the W
pass and no core-to-core exchange is needed. Host sums 8x96x4 partial sums
in float64.

Engine split per min-plus step: ACT computes tmp = src + d^2 (for the W
pass directly from the raw target: tmp = 25*t + d^2 / 25 - 25*t + d^2, which
also absorbs the f-init), DVE does tensor_tensor min in bf16 (2x mode; all
values are small integers, exact in bf16). The H pass shifts partitions,
which compute engines cannot address (start partition must be quadrant-
aligned), so DMA materializes shifted copies of the pre-added volume.
"""

import numpy as np

B, D, H, W = 2, 64, 96, 96
NQ = 4  # W-quarters per batch
WI = W // NQ  # 24 interior columns per core
HALO = 4
WE = WI + 2 * HALO  # 32 columns in the extended slab
N_CORES = B * NQ

_CACHE = {}


def _build():
    import concourse.bacc as bacc
    import concourse.mybir as mybir
    import concourse.tile as tile

    fp32 = mybir.dt.float32
    bf16 = mybir.dt.bfloat16
    AF = mybir.ActivationFunctionType
    ALU = mybir.AluOpType

    nc = bacc.Bacc("TRN2", target_bir_lowering=False, debug=False)
    t_d = nc.dram_tensor("t", [H, D, WE], fp32, kind="ExternalInput").ap()
    p_d = nc.dram_tensor("p", [H, D, WI], fp32, kind="ExternalInput").ap()
    o_d = nc.dram_tensor("o", [H, 4], fp32, kind="ExternalOutput").ap()

    lo, hi = HALO, HALO + WI

    with tile.TileContext(nc) as tc:
        with (
            tc.tile_pool(name="mem", bufs=1) as pool,
            tc.tile_pool(name="tmps", bufs=3) as tpp,
        ):
            ts = pool.tile([H, D, WE], fp32)
            ps = pool.tile([H, D, WI], fp32)
            gw = pool.tile([H, 2, D, WI], bf16)
            gd = pool.tile([H, 2, D, WI], bf16)
            fh = pool.tile([H, 2, D, WI], bf16)
            da = pool.tile([H, 2, D, WI], fp32)
            aw = pool.tile([H, D, WI], fp32)
            lp = pool.tile([H, D, WI], fp32)
            l1p = pool.tile([H, D, WI], fp32)
            tm = pool.tile([H, D, WI], mybir.dt.uint8)
            os_ = pool.tile([H, 4], fp32)

            HD = D // 2  # finalization is pipelined over D-halves

            nc.sync.dma_start(ts[:], t_d)
            nc.gpsimd.dma_start(ps[:], p_d)  # separate queue family from ts

            def tmin(out_ap, in_ap, acc_ap=None):
                nc.vector.tensor_tensor(
                    out_ap, in_ap, acc_ap if acc_ap is not None else out_ap,
                    op=ALU.min,
                )

            tsi = ts[:, :, lo:hi]

            # W pass, fused with the squared-distance init:
            # f_pos = 25*t (t=1 -> "far"=25, t=0 -> 0), f_neg = 25 - 25*t.
            # Inits on ACT while the DVE computes the first pre-add, so the
            # first min starts as early as possible.
            nc.scalar.activation(gw[:, 0], tsi, AF.Copy, scale=25.0)
            nc.scalar.activation(gw[:, 1], tsi, AF.Copy, scale=-25.0, bias=25.0)
            for d in range(1, HALO + 1):
                c = float(d * d)
                # tw covers only the source columns actually read:
                # [lo-d, hi+d) -> 24+2d of the 32
                wd = WI + 2 * d
                tse = ts[:, :, lo - d : hi + d]
                tw = tpp.tile([H, 2, D, wd], bf16, tag=f"tw{d}", name=f"tw{d}",
                              bufs=1)
                if d == 1:
                    # first pre-add on DVE: the first min then skips the
                    # ACT table-load + activation startup chain
                    nc.vector.tensor_scalar(
                        tw[:, 0], tse, 25.0, c, op0=ALU.mult, op1=ALU.add
                    )
                    nc.vector.tensor_scalar(
                        tw[:, 1], tse, -25.0, 25.0 + c, op0=ALU.mult, op1=ALU.add
                    )
                else:
                    nc.scalar.activation(
                        tw[:, 0], tse, AF.Copy, scale=25.0, bias=c
                    )
                    nc.scalar.activation(
                        tw[:, 1], tse, AF.Copy, scale=-25.0, bias=25.0 + c
                    )
                tmin(gw[:], tw[:, :, :, 2 * d :])
                tmin(gw[:], tw[:, :, :, :WI])

            # H pass: gw -> fh. Partition-shifted reads are illegal for
            # compute engines, so DMA materializes shifted copies of the
            # ACT pre-added volume (the up/down shifts of each d go to
            # different queue families so they move in parallel). The -d
            # buffers carry a >=25 sentinel strip in their first d rows;
            # the d=1 downward min is a 3-operand op reading gw directly,
            # which also serves as fh's initialization (no init copy).
            for d in range(1, HALO + 1):
                th_ = tpp.tile([H, 2, D, WI], bf16, tag="t24", name=f"th{d}")
                if d == 1:
                    with tc.high_priority():
                        nc.vector.tensor_scalar(
                            th_[:], gw[:], 1.0, None, op0=ALU.add
                        )
                else:
                    nc.scalar.activation(
                        th_[:], gw[:], AF.Copy, bias=float(d * d)
                    )
                su = tpp.tile([H, 2, D, WI], bf16, tag=f"su{d}", name=f"su{d}",
                              bufs=1)
                sd = tpp.tile([H, 2, D, WI], bf16, tag=f"sd{d}", name=f"sd{d}",
                              bufs=1)
                nc.gpsimd.memset(sd[:d], 100.0)
                nc.sync.dma_start(su[: H - d], th_[d:])
                if d == 1:
                    # 3-operand init (min over the {0,+1} terms) via the
                    # low-latency HWDGE copy; last row gets the plain
                    # delta=0 value, later shifts accumulate into it
                    tmin(fh[: H - 1], su[: H - 1], gw[: H - 1])
                    nc.sync.dma_start(fh[H - 1 :], gw[H - 1 :])
                else:
                    tmin(fh[: H - d], su[: H - d])
                nc.gpsimd.dma_start(sd[d:], th_[: H - d])
                tmin(fh[:], sd[:])

            # D pass: fh -> gd. Same init trick: the first shifted min is
            # 3-operand from fh (plus a cheap row-0 copy for the uncovered
            # row). The d=4 step and everything after run per D-half so
            # the finalization of half 0 overlaps the rest of the D pass.
            for d in range(1, HALO):
                td_ = tpp.tile([H, 2, D, WI], bf16, tag="t24", name=f"td{d}")
                if d == 1:
                    with tc.high_priority():
                        nc.vector.tensor_scalar(
                            td_[:], fh[:], 1.0, None, op0=ALU.add
                        )
                    nc.vector.tensor_copy(gd[:, :, 0:1, :], fh[:, :, 0:1, :])
                    tmin(gd[:, :, 1:, :], td_[:, :, : D - 1, :], fh[:, :, 1:, :])
                    tmin(gd[:, :, : D - 1, :], td_[:, :, 1:, :])
                else:
                    nc.scalar.activation(
                        td_[:], fh[:], AF.Copy, bias=float(d * d)
                    )
                    tmin(gd[:, :, : D - d, :], td_[:, :, d:, :])
                    tmin(gd[:, :, d:, :], td_[:, :, : D - d, :])
            td4 = tpp.tile([H, 2, D, WI], bf16, tag="t24", name="td4")
            nc.scalar.activation(td4[:], fh[:], AF.Copy, bias=16.0)

            # BCE pieces, independent of the EDT: -bce = t*ln(p) +
            # (1-t)*ln(1-p) with logs clamped at -100 (clamp folded into
            # the accumulating op). t is exactly 0/1, so the blend is a
            # predicated copy. The scheduler slots these into engine gaps.
            with tc.tile_wait_until(0.012):  # keep off the W-pass ACT slots
                nc.scalar.activation(lp[:], ps[:], AF.Ln)
                nc.scalar.activation(l1p[:], ps[:], AF.Ln, scale=-1.0, bias=1.0)
            nc.gpsimd.tensor_copy(tm[:], tsi)  # contiguous 0/1 mask
            # gate the blend into the DVE gap where the H pass waits on its
            # first shift DMAs, instead of stealing a pass-critical slot
            with tc.tile_wait_until(0.0256):
                nc.vector.copy_predicated(l1p[:], tm[:], lp[:])  # t?lp:l1p

            # d=4 min + finalization, per D-half:
            # a = sqrt(f_pos)+sqrt(f_neg); w = clamp(2.5-0.5a, 0, 1);
            # o[:,h] = sum_half(w * max(blend, -100)); o[:,2+h] = sum_half(w)
            # per half: da' = 0.5*sqrt(gd) (scale inside the sqrt), then
            # u = clamp(0.5a - 2.5, -1, 0) = -w, with sum(u) = -sum(w)
            # accumulated for free by the clamping tensor_scalar, and
            # o[:,h] = sum(u * max(blend,-100)) = +sum(w*bce).
            for h, (a0, a1) in enumerate(((0, HD), (HD, D))):
                p4 = max(a0, HALO)  # rows with a -4 neighbor
                tmin(gd[:, :, a0 : min(a1, D - HALO), :],
                     td4[:, :, a0 + HALO : min(a1 + HALO, D), :])
                tmin(gd[:, :, p4:a1, :], td4[:, :, p4 - HALO : a1 - HALO, :])
                half = np.s_[:, :, a0:a1, :]
                nc.scalar.activation(da[half], gd[half], AF.Sqrt, scale=0.25)
                awh = aw[:, a0:a1, :]
                nc.vector.scalar_tensor_tensor(
                    awh, da[:, 0, a0:a1, :], 2.5, da[:, 1, a0:a1, :],
                    op0=ALU.subtract, op1=ALU.add,
                )
                nc.vector.tensor_scalar(
                    awh, awh, -1.0, 0.0, op0=ALU.max, op1=ALU.min
                )
                nc.vector.scalar_tensor_tensor(
                    da[:, 0, a0:a1, :], l1p[:, a0:a1, :], -100.0, awh,
                    op0=ALU.max, op1=ALU.mult,
                    accum_out=os_[:, h : h + 1],
                )
                nc.scalar.activation(
                    da[:, 1, a0:a1, :], awh, AF.Copy,
                    accum_out=os_[:, 2 + h : 3 + h],
                )

            for c in range(4):
                nc.sync.dma_start(o_d[:, c : c + 1], os_[:, c : c + 1])
    nc.compile()
    return nc


def _get_nc():
    if "nc" not in _CACHE:
        _CACHE["nc"] = _build()
    return _CACHE["nc"]


def kernel(pred: np.ndarray, target: np.ndarray) -> np.ndarray:
    from concourse.bass_utils import run_bass_kernel_spmd

    nc = _get_nc()

    tp = np.pad(
        np.asarray(target, dtype=np.float32),
        ((0, 0), (0, 0), (0, 0), (HALO, HALO)),
        mode="edge",
    )  # [B, D, H, W + 2*HALO]; edge-replication keeps the EDT exact
    pr = np.asarray(pred, dtype=np.float32)

    in_maps = []
    for b in range(B):
        for q in range(NQ):
            t_slab = np.ascontiguousarray(
                tp[b, :, :, q * WI : q * WI + WE].transpose(1, 0, 2)
            )  # [H, D, WE]
            p_slab = np.ascontiguousarray(
                pr[b, :, :, q * WI : (q + 1) * WI].transpose(1, 0, 2)
            )  # [H, D, WI]
            in_maps.append({"t": t_slab, "p": p_slab})

    res = run_bass_kernel_spmd(nc, in_maps, list(range(N_CORES)))

    loss = 0.0
    for b in range(B):
        num = 0.0
        den = 0.0
        for q in range(NQ):
            s = res.results[b * NQ + q]["o"].sum(axis=0, dtype=np.float64)
            num += s[0] + s[1]
            den -= s[2] + s[3]
        loss += num / (den + 1e-5)
    return np.float32(loss / B)
